# revision 1
# baseline (speedup 1.0000x reference)
"""Trainium2 Bass kernel for a dense transformer encoder layer.

Model (faithful to the oracle):
  q,k,v = x@wq+bq, x@wk+bk, x@wv+bv          (12 heads, dk=64, DIM=768)
  scores = q@k^T / sqrt(768)  (note: sqrt(dim_model), not sqrt(dk))
  scores[mask==0] = 1e-11  (NOT -inf; masked keys still contribute ~1/Z)
  attn = softmax(scores); z = attn@v; o = z@wo+bo
  l1 = x + LN(o);  ffn = relu(l1@w1+b1)@w2+b2;  out = l1 + LN(ffn)

Sharding: 4096 tokens (B=2,S=2048) split 8 ways -> 512 tokens/core.
Cores 0-3 own batch 0, cores 4-7 batch 1. K/V are computed for the
core's whole batch (redundantly within each 4-core group) so attention
needs no collectives.

Softmax trick: scores are built k-major (scoresT [kpos, q]) so the
mask (per-k) is a per-partition scalar; exp(mask_p/sqrt(768) * s) on
the scalar engine applies scale+mask+exp in a single pass (masked rows
give exp(0)=1.0 == fp32(exp(1e-11))). The denominator comes from a
ones column appended to V (attn@v with M=65); normalization happens
after attn@v via a rank-1 matmul broadcast of 1/sum.
"""

import math
import os
import sys

import numpy as np

for _p in ("/opt/trn_rl_repo", os.path.expanduser("~/.axon_site/_ro/trn_rl_repo")):
    if os.path.isdir(_p) and _p not in sys.path:
        sys.path.insert(0, _p)

import ml_dtypes  # noqa: E402

BF16 = ml_dtypes.bfloat16

DIM = 768
HEADS = 12
DK = 64
HID = 4 * DIM  # 3072
B, S = 2, 2048
N_CORES = 8
BLK = 512            # tokens per core
NBLK = S // BLK      # 4 blocks per batch
EPS = 1e-5
ISCALE = 1.0 / math.sqrt(DIM)

_CACHE: dict = {}
MAX_PHASE = int(os.environ.get("BASS_KERNEL_PHASES", "5"))
USE_AG = os.environ.get("BASS_KERNEL_AG", "1") == "1"


def _build_program():
    import concourse.bass as bass
    import concourse.mybir as mybir
    import concourse.tile as tile
    from concourse import bacc
    from concourse.masks import make_identity

    f32 = mybir.dt.float32
    bf16 = mybir.dt.bfloat16
    AF = mybir.ActivationFunctionType
    ALU = mybir.AluOpType
    AX = mybir.AxisListType

    nc = bacc.Bacc()

    # ---- per-core DRAM I/O ----
    if not USE_AG:
        d_xT = nc.dram_tensor("xT", [DIM, S], bf16, kind="ExternalInput")
    d_xTb = nc.dram_tensor("xTb", [DIM, BLK], bf16, kind="ExternalInput")
    d_xb = nc.dram_tensor("xb", [BLK, DIM], f32, kind="ExternalInput")
    d_msc = nc.dram_tensor("msc", [S], f32, kind="ExternalInput")
    d_wq = nc.dram_tensor("wq", [DIM, DIM], bf16, kind="ExternalInput")
    d_wk = nc.dram_tensor("wk", [DIM, DIM], bf16, kind="ExternalInput")
    d_wv = nc.dram_tensor("wv", [DIM, DIM], bf16, kind="ExternalInput")
    d_wo = nc.dram_tensor("wo", [DIM, DIM], bf16, kind="ExternalInput")
    d_w1 = nc.dram_tensor("w1", [DIM, HID], bf16, kind="ExternalInput")
    d_w2 = nc.dram_tensor("w2", [HID, DIM], bf16, kind="ExternalInput")
    d_bq = nc.dram_tensor("bq", [DIM], f32, kind="ExternalInput")
    d_bk = nc.dram_tensor("bk", [DIM], f32, kind="ExternalInput")
    d_bv = nc.dram_tensor("bv", [DIM], f32, kind="ExternalInput")
    d_bo = nc.dram_tensor("bo", [DIM], f32, kind="ExternalInput")
    d_b1 = nc.dram_tensor("b1", [HID], f32, kind="ExternalInput")
    d_b2 = nc.dram_tensor("b2", [DIM], f32, kind="ExternalInput")
    d_g1 = nc.dram_tensor("g1", [DIM], f32, kind="ExternalInput")
    d_bb1 = nc.dram_tensor("bb1", [DIM], f32, kind="ExternalInput")
    d_g2 = nc.dram_tensor("g2", [DIM], f32, kind="ExternalInput")
    d_bb2 = nc.dram_tensor("bb2", [DIM], f32, kind="ExternalInput")
    d_out = nc.dram_tensor("out", [BLK, DIM], f32, kind="ExternalOutput")
    if USE_AG:
        d_kb = nc.dram_tensor("k_bounce", [DIM, BLK], bf16)
        d_ks = nc.dram_tensor("k_shared", [NBLK * DIM, BLK], bf16)
        d_vb = nc.dram_tensor("v_bounce", [BLK, HEADS * (DK + 1)], bf16)
        d_vs = nc.dram_tensor("v_shared", [S, HEADS * (DK + 1)], bf16)
        RG = [[0, 1, 2, 3], [4, 5, 6, 7]]

    FT = DIM // 128   # 6 feature tiles
    TT = BLK // 128   # 4 token tiles per core block
    ST = S // 128     # 16 token tiles per batch
    HT = HID // 128   # 24 hidden tiles

    def bcast_ap(handle, n=128):
        ap = handle[:]
        return bass.AP(tensor=ap.tensor, offset=ap.offset, ap=[[0, n]] + list(ap.ap))

    with tile.TileContext(nc) as tc:
        with (
            tc.tile_pool(name="const", bufs=1) as const,
            tc.tile_pool(name="bigres", bufs=1) as big,
        ):
            # ---------- constants ----------
            sb_msc = const.tile([128, ST], f32)
            nc.sync.dma_start(out=sb_msc, in_=d_msc[:].rearrange("(t p) -> p t", p=128))
            sb_bq = const.tile([128, FT], f32)
            nc.sync.dma_start(out=sb_bq, in_=d_bq[:].rearrange("(t p) -> p t", p=128))
            sb_bk = const.tile([128, FT], f32)
            nc.sync.dma_start(out=sb_bk, in_=d_bk[:].rearrange("(t p) -> p t", p=128))
            sb_b1 = const.tile([128, HT], f32)
            nc.sync.dma_start(out=sb_b1, in_=d_b1[:].rearrange("(t p) -> p t", p=128))
            bv_bc = const.tile([128, DIM], f32)
            nc.gpsimd.dma_start(out=bv_bc, in_=bcast_ap(d_bv))
            bo_bc = const.tile([128, DIM], f32)
            nc.gpsimd.dma_start(out=bo_bc, in_=bcast_ap(d_bo))
            b2_bc = const.tile([128, DIM], f32)
            nc.gpsimd.dma_start(out=b2_bc, in_=bcast_ap(d_b2))
            g1_bc = const.tile([128, DIM], f32)
            nc.gpsimd.dma_start(out=g1_bc, in_=bcast_ap(d_g1))
            bb1_bc = const.tile([128, DIM], f32)
            nc.gpsimd.dma_start(out=bb1_bc, in_=bcast_ap(d_bb1))
            g2_bc = const.tile([128, DIM], f32)
            nc.gpsimd.dma_start(out=g2_bc, in_=bcast_ap(d_g2))
            bb2_bc = const.tile([128, DIM], f32)
            nc.gpsimd.dma_start(out=bb2_bc, in_=bcast_ap(d_bb2))
            ident = const.tile([128, 128], f32)
            make_identity(nc, ident[:])
            ones64 = const.tile([1, 64], f32)
            nc.vector.memset(ones64, 1.0)
            eps_t = const.tile([128, 1], f32)
            nc.vector.memset(eps_t, EPS)

            # ---------- persistent activations ----------
            sb_xblk = big.tile([128, TT, DIM], f32)  # residual x
            sb_l1 = big.tile([128, TT, DIM], f32)

            nc.sync.dma_start(
                out=sb_xblk, in_=d_xb[:].rearrange("(t p) d -> p t d", p=128)
            )

            # attention-scoped residents (freed before the FFN phases)
            attn_res_cm = tc.tile_pool(name="attn_res", bufs=1)
            attn_res = attn_res_cm.__enter__()
            sb_K = attn_res.tile([128, FT, NBLK, BLK], bf16)  # K^T, feat-major
            sb_Q = attn_res.tile([128, FT, BLK], bf16)  # Q^T, feat-major
            sb_V = attn_res.tile([128, ST, HEADS, DK + 1], bf16)  # V + ones col
            sb_zT = attn_res.tile([128, FT, BLK], bf16)  # z^T normalized

            # ============ Phase 1: QKV projections ============
            with (
                tc.tile_pool(name="xw", bufs=1) as xw,
                tc.tile_pool(name="ps1", bufs=4, space="PSUM") as ps1,
                tc.tile_pool(name="ps1v", bufs=4, space="PSUM") as ps1v,
            ):
                if not USE_AG:
                    sb_xT = xw.tile([128, FT, S], bf16)
                    nc.sync.dma_start(
                        out=sb_xT, in_=d_xT[:].rearrange("(t p) n -> p t n", p=128)
                    )
                sb_xTb = xw.tile([128, FT, BLK], bf16)
                nc.sync.dma_start(
                    out=sb_xTb, in_=d_xTb[:].rearrange("(t p) n -> p t n", p=128)
                )
                w_q = xw.tile([128, FT, DIM], bf16)
                nc.sync.dma_start(
                    out=w_q, in_=d_wq[:].rearrange("(t p) o -> p t o", p=128)
                )
                w_k = xw.tile([128, FT, DIM], bf16)
                nc.sync.dma_start(
                    out=w_k, in_=d_wk[:].rearrange("(t p) o -> p t o", p=128)
                )
                w_v = xw.tile([128, FT, DIM], bf16)
                nc.sync.dma_start(
                    out=w_v, in_=d_wv[:].rearrange("(t p) o -> p t o", p=128)
                )

                if USE_AG:
                    # K^T feat-major for the own block only -> bounce -> AG
                    kstage = xw.tile([128, FT, BLK], bf16, tag="kstage")
                    for ft in range(FT):
                        ps = ps1.tile([128, 512], f32, tag="p")
                        for kt in range(FT):
                            nc.tensor.matmul(
                                ps,
                                w_k[:, kt, ft * 128 : (ft + 1) * 128],
                                sb_xTb[:, kt, :],
                                start=(kt == 0),
                                stop=(kt == FT - 1),
                            )
                        nc.vector.tensor_scalar_add(
                            kstage[:, ft, :], ps, sb_bk[:, ft : ft + 1]
                        )
                    nc.sync.dma_start(
                        out=d_kb[:].rearrange("(t p) n -> p t n", p=128), in_=kstage
                    )
                else:
                    # K^T feat-major over the whole batch (replicated)
                    for ft in range(FT):
                        for nt in range(S // 512):
                            ps = ps1.tile([128, 512], f32, tag="p")
                            for kt in range(FT):
                                nc.tensor.matmul(
                                    ps,
                                    w_k[:, kt, ft * 128 : (ft + 1) * 128],
                                    sb_xT[:, kt, nt * 512 : (nt + 1) * 512],
                                    start=(kt == 0),
                                    stop=(kt == FT - 1),
                                )
                            nc.vector.tensor_scalar_add(
                                sb_K[:, ft, nt, :], ps, sb_bk[:, ft : ft + 1]
                            )
                # Q^T feat-major for the core's block
                for ft in range(FT):
                    ps = ps1.tile([128, 512], f32, tag="p")
                    for kt in range(FT):
                        nc.tensor.matmul(
                            ps,
                            w_q[:, kt, ft * 128 : (ft + 1) * 128],
                            sb_xTb[:, kt, :],
                            start=(kt == 0),
                            stop=(kt == FT - 1),
                        )
                    nc.vector.tensor_scalar_add(
                        sb_Q[:, ft, :], ps, sb_bq[:, ft : ft + 1]
                    )
                if USE_AG:
                    # V tok-major for the own block -> bounce -> AG
                    vstage = xw.tile([128, TT, HEADS, DK + 1], bf16, tag="vstage")
                    nc.vector.memset(vstage[:, :, :, DK : DK + 1], 1.0)
                    for tt in range(TT):
                        for nh in range(2):
                            ps = ps1v.tile([128, 384], f32, tag="vp")
                            for kt in range(FT):
                                nc.tensor.matmul(
                                    ps,
                                    sb_xTb[:, kt, tt * 128 : (tt + 1) * 128],
                                    w_v[:, kt, nh * 384 : (nh + 1) * 384],
                                    start=(kt == 0),
                                    stop=(kt == FT - 1),
                                )
                            nc.vector.scalar_tensor_tensor(
                                out=vstage[:, tt, nh * 6 : (nh + 1) * 6, 0:DK],
                                in0=ps[:].rearrange("p (h d) -> p h d", d=DK),
                                scalar=1.0,
                                in1=bv_bc[:, nh * 384 : (nh + 1) * 384].rearrange(
                                    "p (h d) -> p h d", d=DK
                                ),
                                op0=ALU.mult,
                                op1=ALU.add,
                            )
                    nc.sync.dma_start(
                        out=d_vb[:].rearrange("(t p) (h d) -> p t h d", p=128, d=DK + 1),
                        in_=vstage,
                    )
                    # AllGather K and V across the 4-core batch group
                    nc.gpsimd.collective_compute(
                        "AllGather", ALU.bypass, replica_groups=RG,
                        ins=[d_kb[:]], outs=[d_ks[:]],
                    )
                    nc.gpsimd.collective_compute(
                        "AllGather", ALU.bypass, replica_groups=RG,
                        ins=[d_vb[:]], outs=[d_vs[:]],
                    )
                    for b in range(NBLK):
                        nc.sync.dma_start(
                            out=sb_K[:, :, b, :],
                            in_=d_ks[b * DIM : (b + 1) * DIM, :].rearrange(
                                "(t p) n -> p t n", p=128
                            ),
                        )
                    nc.sync.dma_start(
                        out=sb_V,
                        in_=d_vs[:].rearrange(
                            "(t p) (h d) -> p t h d", p=128, d=DK + 1
                        ),
                    )
                else:
                    # V tok-major over the whole batch, laid out [tok, head, dk+1]
                    nc.vector.memset(sb_V[:, :, :, DK : DK + 1], 1.0)
                    for nh in range(2):
                        for tt in range(ST):
                            ps = ps1v.tile([128, 384], f32, tag="vp")
                            for kt in range(FT):
                                nc.tensor.matmul(
                                    ps,
                                    sb_xT[:, kt, tt * 128 : (tt + 1) * 128],
                                    w_v[:, kt, nh * 384 : (nh + 1) * 384],
                                    start=(kt == 0),
                                    stop=(kt == FT - 1),
                                )
                            nc.vector.scalar_tensor_tensor(
                                out=sb_V[:, tt, nh * 6 : (nh + 1) * 6, 0:DK],
                                in0=ps[:].rearrange("p (h d) -> p h d", d=DK),
                                scalar=1.0,
                                in1=bv_bc[:, nh * 384 : (nh + 1) * 384].rearrange(
                                    "p (h d) -> p h d", d=DK
                                ),
                                op0=ALU.mult,
                                op1=ALU.add,
                            )

            if MAX_PHASE >= 2:
                # ============ Phase 2: attention ============
                with (
                    tc.tile_pool(name="expp", bufs=64) as expp,
                    tc.tile_pool(name="attsm", bufs=2) as attsm,
                    tc.tile_pool(name="ps_sc", bufs=4, space="PSUM") as ps_sc,
                    tc.tile_pool(name="ps_z", bufs=2, space="PSUM") as ps_z,
                    tc.tile_pool(name="ps_rb", bufs=1, space="PSUM") as ps_rb,
                ):
                    for hp in range(HEADS // 2):
                        ht = hp
                        # interleave the two heads of a pair kt-by-kt: their
                        # K=64 matmuls sit in disjoint PE row groups (0-63 /
                        # 64-127) so the hardware overlaps adjacent pairs.
                        ets = ([], [])
                        for kt2 in range(ST):
                            for half in (0, 1):
                                ho = half * 64
                                ps = ps_sc.tile([128, BLK], f32, tag="sc")
                                nc.tensor.matmul(
                                    ps,
                                    sb_K[ho : ho + 64, ht, kt2 // 4, (kt2 % 4) * 128 : (kt2 % 4) * 128 + 128],
                                    sb_Q[ho : ho + 64, ht, :],
                                    start=True,
                                    stop=True,
                                )
                                et = expp.tile([128, BLK], bf16, tag="exp")
                                nc.scalar.activation(
                                    et, ps, AF.Exp, scale=sb_msc[:, kt2 : kt2 + 1]
                                )
                                ets[half].append(et)
                        for half in (0, 1):
                            h = 2 * hp + half
                            ho = half * 64
                            zp = ps_z.tile([DK + 1, BLK], f32, tag="z")
                            for kt2 in range(ST):
                                nc.tensor.matmul(
                                    zp,
                                    sb_V[:, kt2, h, :],
                                    ets[half][kt2],
                                    start=(kt2 == 0),
                                    stop=(kt2 == ST - 1),
                                )
                            rsum = attsm.tile([1, BLK], f32, tag="rsum")
                            nc.vector.reciprocal(rsum, zp[DK : DK + 1, :])
                            rbp = ps_rb.tile([64, BLK], f32, tag="rb")
                            nc.tensor.matmul(
                                rbp, ones64[:], rsum, start=True, stop=True
                            )
                            rb = attsm.tile([64, BLK], f32, tag="rbs")
                            nc.vector.tensor_copy(rb, rbp)
                            nc.vector.tensor_mul(
                                sb_zT[ho : ho + 64, ht, :], zp[0:DK, :], rb
                            )

            if MAX_PHASE >= 3:
                # ============ Phase 3: O proj + LN1 (+residual) ============
                def layer_norm_to(out_ap, x_ap, g_bc_t, resid_ap, pool):
                    s = pool.tile([128, 1], f32, tag="ln_s")
                    nc.vector.tensor_reduce(s, x_ap, axis=AX.X, op=ALU.add)
                    mean = pool.tile([128, 1], f32, tag="ln_m")
                    nc.scalar.mul(mean, s, 1.0 / DIM)
                    xc = pool.tile([128, DIM], f32, tag="ln_xc")
                    nc.vector.tensor_scalar(xc, x_ap, mean, None, op0=ALU.subtract)
                    junk = pool.tile([128, DIM], f32, tag="ln_j")
                    var = pool.tile([128, 1], f32, tag="ln_v")
                    # (tensor_tensor_reduce crashes the device on this runtime;
                    # scalar_tensor_tensor with accum_out works)
                    nc.vector.scalar_tensor_tensor(
                        out=junk, in0=xc, scalar=1.0, in1=xc,
                        op0=ALU.mult, op1=ALU.mult, accum_out=var,
                    )
                    nc.vector.tensor_scalar_mul(var, var, 1.0 / DIM)
                    sd = pool.tile([128, 1], f32, tag="ln_sd")
                    nc.scalar.activation(sd, var, AF.Sqrt, bias=eps_t[:])
                    rstd = pool.tile([128, 1], f32, tag="ln_r")
                    nc.vector.reciprocal(rstd, sd)
                    t = pool.tile([128, DIM], f32, tag="ln_t")
                    nc.vector.tensor_scalar(t, xc, rstd, None, op0=ALU.mult)
                    tg = pool.tile([128, DIM], f32, tag="ln_tg")
                    nc.vector.tensor_mul(tg, t, g_bc_t)
                    nc.vector.tensor_add(out_ap, tg, resid_ap)

                with (
                    tc.tile_pool(name="wo_p", bufs=1) as wo_p,
                    tc.tile_pool(name="ln1p", bufs=2) as ln1p,
                    tc.tile_pool(name="ps_o", bufs=4, space="PSUM") as ps_o,
                ):
                    w_o = wo_p.tile([128, FT, DIM], bf16)
                    nc.sync.dma_start(
                        out=w_o, in_=d_wo[:].rearrange("(t p) o -> p t o", p=128)
                    )
                    for tt in range(TT):
                        l1pre = ln1p.tile([128, DIM], f32, tag="l1pre")
                        for nh in range(2):
                            ps = ps_o.tile([128, 384], f32, tag="op")
                            for kt in range(FT):
                                nc.tensor.matmul(
                                    ps,
                                    sb_zT[:, kt, tt * 128 : (tt + 1) * 128],
                                    w_o[:, kt, nh * 384 : (nh + 1) * 384],
                                    start=(kt == 0),
                                    stop=(kt == FT - 1),
                                )
                            nc.vector.scalar_tensor_tensor(
                                out=l1pre[:, nh * 384 : (nh + 1) * 384],
                                in0=ps,
                                scalar=1.0,
                                in1=bo_bc[:, nh * 384 : (nh + 1) * 384],
                                op0=ALU.mult,
                                op1=ALU.add,
                            )
                        xb1 = ln1p.tile([128, DIM], f32, tag="xb1")
                        nc.vector.tensor_add(xb1, sb_xblk[:, tt, :], bb1_bc)
                        layer_norm_to(sb_l1[:, tt, :], l1pre[:], g1_bc, xb1, ln1p)

            attn_res_cm.__exit__(None, None, None)
            sb_hT = big.tile([128, HT, BLK], bf16)  # relu(ffn1)^T, hid-major

            if MAX_PHASE >= 4:
                # ============ Phase 4: transpose l1, FFN1 ============
                with (
                    tc.tile_pool(name="w1_p", bufs=1) as w1_p,
                    tc.tile_pool(name="l1t_p", bufs=1) as l1t_p,
                    tc.tile_pool(name="ps_t", bufs=2, space="PSUM") as ps_t,
                    tc.tile_pool(name="ps_f1", bufs=4, space="PSUM") as ps_f1,
                ):
                    w1_t = []
                    for kt in range(FT):
                        wt = w1_p.tile([128, HID], bf16, tag=f"w1_{kt}")
                        nc.sync.dma_start(
                            out=wt, in_=d_w1[kt * 128 : (kt + 1) * 128, :]
                        )
                        w1_t.append(wt)
                    sb_l1T = l1t_p.tile([128, FT, BLK], bf16)
                    for ft in range(FT):
                        for tt in range(TT):
                            pst = ps_t.tile([128, 128], f32, tag="tp")
                            nc.tensor.transpose(
                                pst, sb_l1[:, tt, ft * 128 : (ft + 1) * 128], ident[:]
                            )
                            nc.scalar.copy(
                                sb_l1T[:, ft, tt * 128 : (tt + 1) * 128], pst
                            )
                    for ht2 in range(HT):
                        ps = ps_f1.tile([128, BLK], f32, tag="f1")
                        for kt in range(FT):
                            nc.tensor.matmul(
                                ps,
                                w1_t[kt][:, ht2 * 128 : (ht2 + 1) * 128],
                                sb_l1T[:, kt, :],
                                start=(kt == 0),
                                stop=(kt == FT - 1),
                            )
                        # relu(x + b1) on DVE: (x add b1) max 0
                        nc.vector.tensor_scalar(
                            sb_hT[:, ht2, :], ps, sb_b1[:, ht2 : ht2 + 1], 0.0,
                            op0=ALU.add, op1=ALU.max,
                        )

            if MAX_PHASE >= 5:
                # ============ Phase 5: FFN2 + LN2 + out ============
                with (
                    tc.tile_pool(name="w2_p", bufs=1) as w2_p,
                    tc.tile_pool(name="ln2p", bufs=2) as ln2p,
                    tc.tile_pool(name="outp", bufs=3) as outp,
                    tc.tile_pool(name="ps_f2", bufs=4, space="PSUM") as ps_f2,
                ):
                    w2_t = []
                    for kt in range(HT):
                        wt = w2_p.tile([128, DIM], bf16, tag=f"w2_{kt}")
                        nc.sync.dma_start(
                            out=wt, in_=d_w2[kt * 128 : (kt + 1) * 128, :]
                        )
                        w2_t.append(wt)
                    out_r = d_out[:].rearrange("(t p) d -> p t d", p=128)
                    for tt in range(TT):
                        f2pre = ln2p.tile([128, DIM], f32, tag="f2pre")
                        for nh in range(2):
                            ps = ps_f2.tile([128, 384], f32, tag="f2")
                            for kt in range(HT):
                                nc.tensor.matmul(
                                    ps,
                                    sb_hT[:, kt, tt * 128 : (tt + 1) * 128],
                                    w2_t[kt][:, nh * 384 : (nh + 1) * 384],
                                    start=(kt == 0),
                                    stop=(kt == HT - 1),
                                )
                            nc.vector.scalar_tensor_tensor(
                                out=f2pre[:, nh * 384 : (nh + 1) * 384],
                                in0=ps,
                                scalar=1.0,
                                in1=b2_bc[:, nh * 384 : (nh + 1) * 384],
                                op0=ALU.mult,
                                op1=ALU.add,
                            )
                        l1b = ln2p.tile([128, DIM], f32, tag="l1b")
                        nc.vector.tensor_add(l1b, sb_l1[:, tt, :], bb2_bc)
                        o_sb = outp.tile([128, DIM], f32, tag="osb")
                        layer_norm_to(o_sb[:], f2pre[:], g2_bc, l1b, ln2p)
                        nc.sync.dma_start(out=out_r[:, tt, :], in_=o_sb)

    return nc


def _get_nc(finalized=True):
    if "nc" not in _CACHE:
        _CACHE["nc"] = _build_program()
    nc = _CACHE["nc"]
    if finalized and not nc.is_finalized():
        nc.finalize()
    return nc


def make_in_maps(inputs: dict) -> list:
    x = np.asarray(inputs["x_n"], np.float32).reshape(B, S, DIM)
    mask = np.asarray(inputs["mask"]).reshape(B, S)
    w = {
        k: np.ascontiguousarray(np.asarray(inputs[k], np.float32).astype(BF16))
        for k in ("wq", "wk", "wv", "wo", "w1", "w2")
    }
    vecs = {
        "bq": inputs["bq"], "bk": inputs["bk"], "bv": inputs["bv"],
        "bo": inputs["bo"], "b1": inputs["b1"], "b2": inputs["b2"],
        "g1": inputs["ln1_g"], "bb1": inputs["ln1_b"],
        "g2": inputs["ln2_g"], "bb2": inputs["ln2_b"],
    }
    vecs = {k: np.ascontiguousarray(np.asarray(v, np.float32)) for k, v in vecs.items()}
    in_maps = []
    for c in range(N_CORES):
        b, blk = c // NBLK, c % NBLK
        xb = x[b]
        xT = None if USE_AG else np.ascontiguousarray(xb.T.astype(BF16))
        xblk = np.ascontiguousarray(xb[blk * BLK : (blk + 1) * BLK])
        xTb = np.ascontiguousarray(xblk.T.astype(BF16))
        msc = (mask[b].astype(np.float32) != 0).astype(np.float32) * ISCALE
        m = {"xTb": xTb, "xb": xblk, "msc": msc}
        if not USE_AG:
            m["xT"] = xT
        m.update(w)
        m.update(vecs)
        in_maps.append(m)
    return in_maps


def assemble(per_core_out: list) -> np.ndarray:
    blocks = [np.asarray(o, np.float32) for o in per_core_out]
    full = np.concatenate(blocks, axis=0).reshape(B, S, DIM)
    return full


def kernel(**inputs) -> np.ndarray:
    from concourse.bass_utils import run_bass_kernel_spmd

    nc = _get_nc()
    in_maps = make_in_maps(inputs)
    res = run_bass_kernel_spmd(nc, in_maps, list(range(N_CORES)))
    return assemble([r["out"] for r in res.results])



# revision 11
# speedup vs baseline: 1.2284x; 1.2284x over previous
"""Trainium2 Bass kernel for a dense transformer encoder layer (v2).

Model (faithful to the oracle):
  q,k,v = x@wq+bq, x@wk+bk, x@wv+bv          (12 heads, dk=64, DIM=768)
  scores = q@k^T / sqrt(768)  (note: sqrt(dim_model), not sqrt(dk))
  scores[mask==0] = 1e-11  (NOT -inf; masked keys contribute exp(1e-11)=1)
  attn = softmax(scores); z = attn@v; o = z@wo+bo
  l1 = x + LN(o);  ffn = relu(l1@w1+b1)@w2+b2;  out = l1 + LN(ffn)

Sharding: 4096 tokens (B=2,S=2048) split 8 ways -> 512 tokens/core.
Cores 0-3 own batch 0, cores 4-7 batch 1. Each core projects K/V for its
own 512-token block; two AllGathers (K then V) within each 4-core batch
group replicate them; both overlap with Q projection and the QK^T+exp
pipeline (scores only need K, attn@V starts once V lands).

Perf notes (from NTFF traces of v1):
 - matmul cost ~ moving-dim cols; consecutive matmuls must hit different
   PSUM banks to pipeline (~195ns/384col vs ~427ns serialized).
 - scores use packed pair K tiles (two heads per 128 partitions) with
   per-head zero-padded Q copies, so contraction is a full 128 rows.
 - mask folded into et post-exp (et' = m*et + (1-m)) so exp scale is a
   compile-time constant and activations batch 2 k-tiles per instr.
 - softmax denominators: ones column in V; 1/den via fast DVE reciprocal,
   broadcast across partitions with a rank-1 matmul.
"""

import math
import os
import sys

import numpy as np

for _p in ("/opt/trn_rl_repo", os.path.expanduser("~/.axon_site/_ro/trn_rl_repo")):
    if os.path.isdir(_p) and _p not in sys.path:
        sys.path.insert(0, _p)

import ml_dtypes  # noqa: E402

BF16 = ml_dtypes.bfloat16

DIM = 768
HEADS = 12
DK = 64
HID = 4 * DIM  # 3072
B, S = 2, 2048
N_CORES = 8
BLK = 512            # tokens per core
NBLK = S // BLK      # 4 blocks per batch
EPS = 1e-5
ISCALE = 1.0 / math.sqrt(DIM)

FT = DIM // 128   # 6 feature tiles
TT = BLK // 128   # 4 token tiles per core block
ST = S // 128     # 16 key tiles per batch
HT = HID // 128   # 24 hidden tiles
NG = ST // 2      # 8 exp groups (2 k-tiles each) per head

_CACHE: dict = {}
DBG = os.environ.get("BASS_DEBUG", "0") == "1"


def _build_program():
    import concourse.bass as bass
    import concourse.mybir as mybir
    import concourse.tile as tile
    from concourse import bacc
    from concourse.masks import make_identity

    f32 = mybir.dt.float32
    bf16 = mybir.dt.bfloat16
    AF = mybir.ActivationFunctionType
    ALU = mybir.AluOpType
    AX = mybir.AxisListType

    nc = bacc.Bacc()

    # ---- per-core DRAM I/O ----
    d_xTb = nc.dram_tensor("xTb", [DIM, BLK], bf16, kind="ExternalInput")
    d_xb = nc.dram_tensor("xb", [BLK, DIM], f32, kind="ExternalInput")
    d_m01 = nc.dram_tensor("m01", [S], f32, kind="ExternalInput")
    d_m1m = nc.dram_tensor("m1m", [S], f32, kind="ExternalInput")
    d_wq = nc.dram_tensor("wq", [DIM, DIM], bf16, kind="ExternalInput")
    d_wk = nc.dram_tensor("wk", [DIM, DIM], bf16, kind="ExternalInput")
    d_wv = nc.dram_tensor("wv", [DIM, DIM], bf16, kind="ExternalInput")
    d_wo = nc.dram_tensor("wo", [DIM, DIM], bf16, kind="ExternalInput")
    d_w1 = nc.dram_tensor("w1", [DIM, HID], bf16, kind="ExternalInput")
    d_w2 = nc.dram_tensor("w2", [HID, DIM], bf16, kind="ExternalInput")
    d_bq = nc.dram_tensor("bq", [DIM], f32, kind="ExternalInput")
    d_bk = nc.dram_tensor("bk", [DIM], f32, kind="ExternalInput")
    d_bv = nc.dram_tensor("bv", [DIM], f32, kind="ExternalInput")
    d_bo = nc.dram_tensor("bo", [DIM], f32, kind="ExternalInput")
    d_b1 = nc.dram_tensor("b1", [HID], f32, kind="ExternalInput")
    d_b2 = nc.dram_tensor("b2", [DIM], f32, kind="ExternalInput")
    d_g1 = nc.dram_tensor("g1", [DIM], f32, kind="ExternalInput")
    d_bb1 = nc.dram_tensor("bb1", [DIM], f32, kind="ExternalInput")
    d_g2 = nc.dram_tensor("g2", [DIM], f32, kind="ExternalInput")
    d_bb2 = nc.dram_tensor("bb2", [DIM], f32, kind="ExternalInput")
    d_out = nc.dram_tensor("out", [BLK, DIM], f32, kind="ExternalOutput")
    if DBG:
        d_dbg_k = nc.dram_tensor("dbg_k", [DIM, BLK], bf16, kind="ExternalOutput")
        d_dbg_q = nc.dram_tensor("dbg_q", [128, HEADS, BLK], bf16, kind="ExternalOutput")
        d_dbg_et = nc.dram_tensor("dbg_et", [128, 2, BLK], bf16, kind="ExternalOutput")
        d_dbg_zt = nc.dram_tensor("dbg_zt", [DIM, BLK], bf16, kind="ExternalOutput")
        d_dbg_l1 = nc.dram_tensor("dbg_l1", [128, TT, DIM], f32, kind="ExternalOutput")
        d_dbg_l1t = nc.dram_tensor("dbg_l1t", [DIM, BLK], bf16, kind="ExternalOutput")
        d_dbg_ht = nc.dram_tensor("dbg_ht", [128, HT, BLK], bf16, kind="ExternalOutput")
        d_dbg_z0 = nc.dram_tensor("dbg_z0", [DK + 1, BLK], f32, kind="ExternalOutput")
        d_dbg_rd = nc.dram_tensor("dbg_rd", [1, BLK], f32, kind="ExternalOutput")
        d_dbg_rb = nc.dram_tensor("dbg_rb", [DK, BLK], f32, kind="ExternalOutput")
    d_kb = nc.dram_tensor("k_bounce", [DIM, BLK], bf16)
    d_ks = nc.dram_tensor("k_shared", [NBLK * DIM, BLK], bf16)
    d_vb = nc.dram_tensor("v_bounce", [BLK, HEADS * (DK + 1)], bf16)
    d_vs = nc.dram_tensor("v_shared", [S, HEADS * (DK + 1)], bf16)
    RG = [[0, 1, 2, 3], [4, 5, 6, 7]]

    def bcast_ap(handle, n=128):
        ap = handle[:]
        return bass.AP(tensor=ap.tensor, offset=ap.offset, ap=[[0, n]] + list(ap.ap))

    with tile.TileContext(nc) as tc:
        with (
            tc.tile_pool(name="const", bufs=1) as const,
            tc.tile_pool(name="bigres", bufs=1) as big,
        ):
            # ---------- constants ----------
            sb_m01 = const.tile([128, ST], f32)
            nc.gpsimd.dma_start(out=sb_m01, in_=d_m01[:].rearrange("(t p) -> p t", p=128))
            sb_m1m = const.tile([128, ST], f32)
            nc.gpsimd.dma_start(out=sb_m1m, in_=d_m1m[:].rearrange("(t p) -> p t", p=128))
            sb_bq = const.tile([128, FT], f32)
            nc.gpsimd.dma_start(out=sb_bq, in_=d_bq[:].rearrange("(t p) -> p t", p=128))
            sb_bk = const.tile([128, FT], f32)
            nc.gpsimd.dma_start(out=sb_bk, in_=d_bk[:].rearrange("(t p) -> p t", p=128))
            sb_b1 = const.tile([128, HT], f32)
            nc.gpsimd.dma_start(out=sb_b1, in_=d_b1[:].rearrange("(t p) -> p t", p=128))
            bv_bc = const.tile([128, DIM], f32)
            nc.gpsimd.dma_start(out=bv_bc, in_=bcast_ap(d_bv))
            bo_bc = const.tile([128, DIM], f32)
            nc.gpsimd.dma_start(out=bo_bc, in_=bcast_ap(d_bo))
            b2_bc = const.tile([128, DIM], f32)
            nc.gpsimd.dma_start(out=b2_bc, in_=bcast_ap(d_b2))
            g1_bc = const.tile([128, DIM], f32)
            nc.gpsimd.dma_start(out=g1_bc, in_=bcast_ap(d_g1))
            bb1_bc = const.tile([128, DIM], f32)
            nc.gpsimd.dma_start(out=bb1_bc, in_=bcast_ap(d_bb1))
            g2_bc = const.tile([128, DIM], f32)
            nc.gpsimd.dma_start(out=g2_bc, in_=bcast_ap(d_g2))
            bb2_bc = const.tile([128, DIM], f32)
            nc.gpsimd.dma_start(out=bb2_bc, in_=bcast_ap(d_bb2))
            ident_bf = const.tile([128, 128], bf16)
            make_identity(nc, ident_bf[:])
            ones_t = const.tile([128, DK], f32)
            nc.vector.memset(ones_t, 1.0)
            eps_t = const.tile([128, 1], f32)
            nc.vector.memset(eps_t, EPS)

            # ---------- persistent activations ----------
            sb_xblk = big.tile([128, TT, DIM], f32)  # residual x
            sb_l1 = big.tile([128, TT, DIM], f32)
            nc.scalar.dma_start(
                out=sb_xblk, in_=d_xb[:].rearrange("(t p) d -> p t d", p=128)
            )

            # ffn-scoped residents (l1T consumed by FFN1, hT by FFN2);
            # opened first so later pools can close in LIFO order
            ffn_res_cm = tc.tile_pool(name="ffn_res", bufs=1)
            ffn_res = ffn_res_cm.__enter__()
            sb_l1T = ffn_res.tile([128, FT, BLK], bf16)
            sb_hT = ffn_res.tile([128, HT, BLK], bf16)

            # wo loaded early (consumed in phase 3)
            wo_cm = tc.tile_pool(name="wo_p", bufs=1)
            wo_p = wo_cm.__enter__()
            w_o = wo_p.tile([128, FT, DIM], bf16)
            nc.scalar.dma_start(
                out=w_o, in_=d_wo[:].rearrange("(t p) o -> p t o", p=128)
            )

            # attention-scoped residents (freed after phase 3's O-projection)
            attn_res_cm = tc.tile_pool(name="attn_res", bufs=1)
            attn_res = attn_res_cm.__enter__()
            sb_K = attn_res.tile([128, FT, NBLK, BLK], bf16)  # K^T, feat-major
            sb_Qp = attn_res.tile([128, HEADS, BLK], bf16)  # per-head padded Q^T
            sb_V = attn_res.tile([128, ST, HEADS, DK + 1], bf16)  # V + ones col
            sb_zT = attn_res.tile([128, FT, BLK], bf16)  # z^T normalized
            nc.vector.memset(sb_Qp, 0.0)

            # ============ Phase 1: QKV projections + AllGathers ============
            with (
                tc.tile_pool(name="xw", bufs=1) as xw,
                tc.tile_pool(name="ps1", bufs=4, space="PSUM") as ps1,
                tc.tile_pool(name="ps1v", bufs=4, space="PSUM") as ps1v,
            ):
                sb_xTb = xw.tile([128, FT, BLK], bf16)
                nc.sync.dma_start(
                    out=sb_xTb, in_=d_xTb[:].rearrange("(t p) n -> p t n", p=128)
                )
                w_k = xw.tile([128, FT, DIM], bf16)
                nc.sync.dma_start(
                    out=w_k, in_=d_wk[:].rearrange("(t p) o -> p t o", p=128)
                )
                w_v = xw.tile([128, FT, DIM], bf16)
                nc.sync.dma_start(
                    out=w_v, in_=d_wv[:].rearrange("(t p) o -> p t o", p=128)
                )
                w_q = xw.tile([128, FT, DIM], bf16)
                nc.sync.dma_start(
                    out=w_q, in_=d_wq[:].rearrange("(t p) o -> p t o", p=128)
                )

                # K^T feat-major for the own block; interleave 3 psum chains
                kstage = xw.tile([128, FT, BLK], bf16, tag="kstage")
                for g in range(2):
                    pss = [ps1.tile([128, BLK], f32, tag="p", name="p") for _ in range(3)]
                    for kt in range(FT):
                        for j in range(3):
                            ft = g * 3 + j
                            nc.tensor.matmul(
                                pss[j],
                                w_k[:, kt, ft * 128 : (ft + 1) * 128],
                                sb_xTb[:, kt, :],
                                start=(kt == 0),
                                stop=(kt == FT - 1),
                            )
                    for j in range(3):
                        ft = g * 3 + j
                        nc.vector.tensor_scalar_add(
                            kstage[:, ft, :], pss[j], sb_bk[:, ft : ft + 1]
                        )
                nc.sync.dma_start(
                    out=d_kb[:].rearrange("(t p) n -> p t n", p=128), in_=kstage
                )
                nc.gpsimd.collective_compute(
                    "AllGather", ALU.bypass, replica_groups=RG,
                    ins=[d_kb[:]], outs=[d_ks[:]],
                )

                # V tok-major for the own block -> bounce -> AG
                vstage = xw.tile([128, TT, HEADS, DK + 1], bf16, tag="vstage")
                nc.vector.memset(vstage[:, :, :, DK : DK + 1], 1.0)
                vchunks = [(tt, nh) for tt in range(TT) for nh in range(2)]
                for g in range(2):
                    batch = vchunks[g * 4 : (g + 1) * 4]
                    pss = [ps1v.tile([128, 384], f32, tag="vp", name="vp") for _ in batch]
                    for kt in range(FT):
                        for j, (tt, nh) in enumerate(batch):
                            nc.tensor.matmul(
                                pss[j],
                                sb_xTb[:, kt, tt * 128 : (tt + 1) * 128],
                                w_v[:, kt, nh * 384 : (nh + 1) * 384],
                                start=(kt == 0),
                                stop=(kt == FT - 1),
                            )
                    for j, (tt, nh) in enumerate(batch):
                        nc.vector.scalar_tensor_tensor(
                            out=vstage[:, tt, nh * 6 : (nh + 1) * 6, 0:DK],
                            in0=pss[j][:].rearrange("p (h d) -> p h d", d=DK),
                            scalar=1.0,
                            in1=bv_bc[:, nh * 384 : (nh + 1) * 384].rearrange(
                                "p (h d) -> p h d", d=DK
                            ),
                            op0=ALU.mult,
                            op1=ALU.add,
                        )
                nc.sync.dma_start(
                    out=d_vb[:].rearrange("(t p) (h d) -> p t h d", p=128, d=DK + 1),
                    in_=vstage,
                )
                nc.gpsimd.collective_compute(
                    "AllGather", ALU.bypass, replica_groups=RG,
                    ins=[d_vb[:]], outs=[d_vs[:]],
                )

                # Q^T feat-major; write per-head zero-padded copies
                for g in range(2):
                    pss = [ps1.tile([128, BLK], f32, tag="p", name="p") for _ in range(3)]
                    for kt in range(FT):
                        for j in range(3):
                            ft = g * 3 + j
                            nc.tensor.matmul(
                                pss[j],
                                w_q[:, kt, ft * 128 : (ft + 1) * 128],
                                sb_xTb[:, kt, :],
                                start=(kt == 0),
                                stop=(kt == FT - 1),
                            )
                    for j in range(3):
                        ft = g * 3 + j
                        nc.vector.tensor_scalar_add(
                            sb_Qp[0:64, 2 * ft, :], pss[j][0:64, :],
                            sb_bq[0:64, ft : ft + 1],
                        )
                        nc.vector.tensor_scalar_add(
                            sb_Qp[64:128, 2 * ft + 1, :], pss[j][64:128, :],
                            sb_bq[64:128, ft : ft + 1],
                        )

                if DBG:
                    nc.sync.dma_start(
                        out=d_dbg_k[:].rearrange("(t p) n -> p t n", p=128),
                        in_=kstage,
                    )
                    nc.sync.dma_start(out=d_dbg_q[:], in_=sb_Qp)
                # unpack the gathered K/V (waits on the collectives)
                for b in range(NBLK):
                    nc.sync.dma_start(
                        out=sb_K[:, :, b, :],
                        in_=d_ks[b * DIM : (b + 1) * DIM, :].rearrange(
                            "(t p) n -> p t n", p=128
                        ),
                    )
                nc.sync.dma_start(
                    out=sb_V,
                    in_=d_vs[:].rearrange(
                        "(t p) (h d) -> p t h d", p=128, d=DK + 1
                    ),
                )

            # ============ Phase 2: attention ============
            with (
                tc.tile_pool(name="expp", bufs=20) as expp,
                tc.tile_pool(name="normp", bufs=2) as normp,
                tc.tile_pool(name="ps_sc", bufs=2, space="PSUM") as ps_sc,
                tc.tile_pool(name="ps_z", bufs=2, space="PSUM") as ps_z,
                tc.tile_pool(name="ps_rb", bufs=1, space="PSUM") as ps_rb,
            ):
                et_tiles = {}

                def emit_scores_group(h, g):
                    ps = ps_sc.tile([128, 2, BLK], f32, tag="sc")
                    for j in (0, 1):
                        kt2 = 2 * g + j
                        blk, col = kt2 // NBLK, kt2 % NBLK
                        nc.tensor.matmul(
                            ps[:, j, :],
                            sb_K[:, h // 2, blk, col * 128 : (col + 1) * 128],
                            sb_Qp[:, h, :],
                            start=True, stop=True, skip_group_check=True,
                        )
                    et = expp.tile([128, 2, BLK], bf16, tag="et")
                    nc.scalar.activation(et, ps, AF.Exp, scale=ISCALE)
                    for j in (0, 1):
                        kt2 = 2 * g + j
                        nc.vector.tensor_scalar(
                            et[:, j, :], et[:, j, :],
                            sb_m01[:, kt2 : kt2 + 1], sb_m1m[:, kt2 : kt2 + 1],
                            op0=ALU.mult, op1=ALU.add,
                        )
                    if DBG and h == 0 and g == 0:
                        nc.sync.dma_start(out=d_dbg_et[:], in_=et)
                    et_tiles[(h, g)] = et

                def emit_attnv_steps(h, zp, g):
                    et = et_tiles.pop((h, g))
                    for j in (0, 1):
                        kt2 = 2 * g + j
                        nc.tensor.matmul(
                            zp,
                            sb_V[:, kt2, h, :],
                            et[:, j, :],
                            start=(kt2 == 0), stop=(kt2 == ST - 1),
                            skip_group_check=True,
                        )

                LOOKAHEAD = 2
                for h in range(LOOKAHEAD):
                    for g in range(NG):
                        emit_scores_group(h, g)
                for h in range(HEADS):
                    zp = ps_z.tile([DK + 1, BLK], f32, tag="z")
                    for g in range(NG):
                        if h + LOOKAHEAD < HEADS:
                            emit_scores_group(h + LOOKAHEAD, g)
                        emit_attnv_steps(h, zp, g)
                    # softmax normalization: z /= den (den = ones-col of V)
                    ht = h // 2
                    ho = (h % 2) * 64
                    if DBG and h == 0:
                        z0dbg = normp.tile([DK + 1, BLK], f32, tag="z0dbg")
                        nc.vector.tensor_copy(z0dbg, zp)
                        nc.sync.dma_start(out=d_dbg_z0[:], in_=z0dbg)
                    den_sb = normp.tile([1, BLK], f32, tag="den_sb")
                    nc.vector.tensor_copy(den_sb, zp[DK : DK + 1, :])
                    rden = normp.tile([128, BLK], f32, tag="rden")
                    nc.vector.reciprocal_approx_fast(rden[0:1, :], den_sb)
                    rb = ps_rb.tile([DK, BLK], f32, tag="rb")
                    nc.tensor.matmul(
                        rb, ones_t[0:1, :], rden[0:1, :],
                        start=True, stop=True, skip_group_check=True,
                    )
                    rbs = normp.tile([DK, BLK], f32, tag="rbs")
                    nc.vector.tensor_copy(rbs, rb)
                    if DBG and h == 0:
                        nc.sync.dma_start(out=d_dbg_rd[:], in_=rden[0:1, :])
                        nc.sync.dma_start(out=d_dbg_rb[:], in_=rbs)
                    nc.vector.tensor_mul(
                        sb_zT[ho : ho + 64, ht, :], zp[0:DK, :], rbs
                    )

            if DBG:
                nc.sync.dma_start(
                    out=d_dbg_zt[:].rearrange("(t p) n -> p t n", p=128), in_=sb_zT
                )

            # ============ Phase 3: O proj + LN1 (+residual) ============
            def layer_norm_to(out_ap, x_ap, g_bc_t, resid_ap, pool):
                s = pool.tile([128, 1], f32, tag="ln_s")
                nc.vector.tensor_reduce(s, x_ap, axis=AX.X, op=ALU.add)
                mean = pool.tile([128, 1], f32, tag="ln_m")
                nc.scalar.mul(mean, s, 1.0 / DIM)
                xc = pool.tile([128, DIM], f32, tag="ln_xc")
                nc.vector.tensor_scalar(xc, x_ap, mean, None, op0=ALU.subtract)
                junk = pool.tile([128, DIM], f32, tag="ln_j")
                var = pool.tile([128, 1], f32, tag="ln_v")
                # (tensor_tensor_reduce crashes the device on this runtime;
                # scalar_tensor_tensor with accum_out works)
                nc.vector.scalar_tensor_tensor(
                    out=junk, in0=xc, scalar=1.0, in1=xc,
                    op0=ALU.mult, op1=ALU.mult, accum_out=var,
                )
                nc.vector.tensor_scalar_mul(var, var, 1.0 / DIM)
                sd = pool.tile([128, 1], f32, tag="ln_sd")
                nc.scalar.activation(sd, var, AF.Sqrt, bias=eps_t[:])
                rstd = pool.tile([128, 1], f32, tag="ln_r")
                nc.vector.reciprocal(rstd, sd)
                t = pool.tile([128, DIM], f32, tag="ln_t")
                nc.vector.tensor_scalar(t, xc, rstd, None, op0=ALU.mult)
                tg = pool.tile([128, DIM], f32, tag="ln_tg")
                nc.vector.tensor_mul(tg, t, g_bc_t)
                nc.vector.tensor_add(out_ap, tg, resid_ap)

            with (
                tc.tile_pool(name="ln1p", bufs=2) as ln1p,
                tc.tile_pool(name="l1bp", bufs=1) as l1bp,
                tc.tile_pool(name="ps_o", bufs=4, space="PSUM") as ps_o,
                tc.tile_pool(name="ps_t", bufs=2, space="PSUM") as ps_t,
            ):
                sb_l1b = l1bp.tile([128, TT, DIM], bf16)
                ochunks = [(tt, nh) for tt in range(TT) for nh in range(2)]
                l1pres = {}
                for gb in range(2):
                    batch = ochunks[gb * 4 : (gb + 1) * 4]
                    pss = [ps_o.tile([128, 384], f32, tag="op", name="op") for _ in batch]
                    for kt in range(FT):
                        for j, (tt, nh) in enumerate(batch):
                            nc.tensor.matmul(
                                pss[j],
                                sb_zT[:, kt, tt * 128 : (tt + 1) * 128],
                                w_o[:, kt, nh * 384 : (nh + 1) * 384],
                                start=(kt == 0),
                                stop=(kt == FT - 1),
                            )
                    for j, (tt, nh) in enumerate(batch):
                        if tt not in l1pres:
                            l1pres[tt] = ln1p.tile([128, DIM], f32, tag="l1pre", name="l1pre")
                        nc.vector.scalar_tensor_tensor(
                            out=l1pres[tt][:, nh * 384 : (nh + 1) * 384],
                            in0=pss[j],
                            scalar=1.0,
                            in1=bo_bc[:, nh * 384 : (nh + 1) * 384],
                            op0=ALU.mult,
                            op1=ALU.add,
                        )
                    for tt, nh in batch:
                        if nh != 1:
                            continue
                        xb1 = ln1p.tile([128, DIM], f32, tag="xb1")
                        nc.vector.tensor_add(xb1, sb_xblk[:, tt, :], bb1_bc)
                        layer_norm_to(
                            sb_l1[:, tt, :], l1pres.pop(tt)[:], g1_bc, xb1, ln1p
                        )
                        nc.scalar.copy(sb_l1b[:, tt, :], sb_l1[:, tt, :])
                        for ft in range(FT):
                            pst = ps_t.tile([128, 128], bf16, tag="tp")
                            nc.tensor.transpose(
                                pst, sb_l1b[:, tt, ft * 128 : (ft + 1) * 128],
                                ident_bf[:],
                            )
                            nc.vector.tensor_copy(
                                sb_l1T[:, ft, tt * 128 : (tt + 1) * 128], pst
                            )

            if DBG:
                nc.sync.dma_start(out=d_dbg_l1[:], in_=sb_l1)
                nc.sync.dma_start(
                    out=d_dbg_l1t[:].rearrange("(t p) n -> p t n", p=128),
                    in_=sb_l1T,
                )
            attn_res_cm.__exit__(None, None, None)
            wo_cm.__exit__(None, None, None)

            # ============ Phase 4: FFN1 ============
            w2_cm = tc.tile_pool(name="w2_p", bufs=1)
            w2_p = w2_cm.__enter__()
            w1_cm = tc.tile_pool(name="w1_p", bufs=1)
            w1_p = w1_cm.__enter__()
            w1_t = []
            for kt in range(FT):
                wt = w1_p.tile([128, HID], bf16, tag=f"w1_{kt}", name=f"w1_{kt}")
                nc.sync.dma_start(out=wt, in_=d_w1[kt * 128 : (kt + 1) * 128, :])
                w1_t.append(wt)
            w2_t = []
            for kt in range(HT):
                wt = w2_p.tile([128, DIM], bf16, tag=f"w2_{kt}", name=f"w2_{kt}")
                nc.sync.dma_start(out=wt, in_=d_w2[kt * 128 : (kt + 1) * 128, :])
                w2_t.append(wt)

            with tc.tile_pool(name="ps_f1", bufs=4, space="PSUM") as ps_f1:
                for g in range(0, HT, 3):
                    pss = [ps_f1.tile([128, BLK], f32, tag="f1", name="f1") for _ in range(3)]
                    for kt in range(FT):
                        for j in range(3):
                            ht2 = g + j
                            nc.tensor.matmul(
                                pss[j],
                                w1_t[kt][:, ht2 * 128 : (ht2 + 1) * 128],
                                sb_l1T[:, kt, :],
                                start=(kt == 0),
                                stop=(kt == FT - 1),
                            )
                    for j in range(3):
                        ht2 = g + j
                        # relu(x + b1) on DVE: (x add b1) max 0
                        nc.vector.tensor_scalar(
                            sb_hT[:, ht2, :], pss[j], sb_b1[:, ht2 : ht2 + 1], 0.0,
                            op0=ALU.add, op1=ALU.max,
                        )
            w1_cm.__exit__(None, None, None)

            if DBG:
                nc.sync.dma_start(out=d_dbg_ht[:], in_=sb_hT)

            # ============ Phase 5: FFN2 + LN2 + out ============
            with (
                tc.tile_pool(name="ln2p", bufs=2) as ln2p,
                tc.tile_pool(name="outp", bufs=3) as outp,
                tc.tile_pool(name="ps_f2", bufs=4, space="PSUM") as ps_f2,
            ):
                out_r = d_out[:].rearrange("(t p) d -> p t d", p=128)
                for tt in range(TT):
                    f2pre = ln2p.tile([128, DIM], f32, tag="f2pre")
                    for nh in range(2):
                        ps = ps_f2.tile([128, 384], f32, tag="f2")
                        for kt in range(HT):
                            nc.tensor.matmul(
                                ps,
                                sb_hT[:, kt, tt * 128 : (tt + 1) * 128],
                                w2_t[kt][:, nh * 384 : (nh + 1) * 384],
                                start=(kt == 0),
                                stop=(kt == HT - 1),
                            )
                        nc.vector.scalar_tensor_tensor(
                            out=f2pre[:, nh * 384 : (nh + 1) * 384],
                            in0=ps,
                            scalar=1.0,
                            in1=b2_bc[:, nh * 384 : (nh + 1) * 384],
                            op0=ALU.mult,
                            op1=ALU.add,
                        )
                    l1b2 = ln2p.tile([128, DIM], f32, tag="l1b2")
                    nc.vector.tensor_add(l1b2, sb_l1[:, tt, :], bb2_bc)
                    o_sb = outp.tile([128, DIM], f32, tag="osb")
                    layer_norm_to(o_sb[:], f2pre[:], g2_bc, l1b2, ln2p)
                    nc.sync.dma_start(out=out_r[:, tt, :], in_=o_sb)
            w2_cm.__exit__(None, None, None)
            ffn_res_cm.__exit__(None, None, None)

    return nc


def _get_nc(finalized=True):
    if "nc" not in _CACHE:
        _CACHE["nc"] = _build_program()
    nc = _CACHE["nc"]
    if finalized and not nc.is_finalized():
        nc.finalize()
    return nc


def make_in_maps(inputs: dict) -> list:
    x = np.asarray(inputs["x_n"], np.float32).reshape(B, S, DIM)
    mask = np.asarray(inputs["mask"]).reshape(B, S)
    w = {
        k: np.ascontiguousarray(np.asarray(inputs[k], np.float32).astype(BF16))
        for k in ("wq", "wk", "wv", "wo", "w1", "w2")
    }
    vecs = {
        "bq": inputs["bq"], "bk": inputs["bk"], "bv": inputs["bv"],
        "bo": inputs["bo"], "b1": inputs["b1"], "b2": inputs["b2"],
        "g1": inputs["ln1_g"], "bb1": inputs["ln1_b"],
        "g2": inputs["ln2_g"], "bb2": inputs["ln2_b"],
    }
    vecs = {k: np.ascontiguousarray(np.asarray(v, np.float32)) for k, v in vecs.items()}
    in_maps = []
    for c in range(N_CORES):
        b, blk = c // NBLK, c % NBLK
        xb = x[b]
        xblk = np.ascontiguousarray(xb[blk * BLK : (blk + 1) * BLK])
        xTb = np.ascontiguousarray(xblk.T.astype(BF16))
        m01 = (mask[b] != 0).astype(np.float32)
        m1m = np.float32(1.0) - m01
        m = {"xTb": xTb, "xb": xblk, "m01": m01, "m1m": m1m}
        m.update(w)
        m.update(vecs)
        in_maps.append(m)
    return in_maps


def assemble(per_core_out: list) -> np.ndarray:
    blocks = [np.asarray(o, np.float32) for o in per_core_out]
    full = np.concatenate(blocks, axis=0).reshape(B, S, DIM)
    return full


def kernel(**inputs) -> np.ndarray:
    from concourse.bass_utils import run_bass_kernel_spmd

    nc = _get_nc()
    in_maps = make_in_maps(inputs)
    res = run_bass_kernel_spmd(nc, in_maps, list(range(N_CORES)))
    return assemble([r["out"] for r in res.results])


# revision 14
# speedup vs baseline: 1.3477x; 1.0971x over previous
"""Trainium2 Bass kernel for a dense transformer encoder layer (v2).

Model (faithful to the oracle):
  q,k,v = x@wq+bq, x@wk+bk, x@wv+bv          (12 heads, dk=64, DIM=768)
  scores = q@k^T / sqrt(768)  (note: sqrt(dim_model), not sqrt(dk))
  scores[mask==0] = 1e-11  (NOT -inf; masked keys contribute exp(1e-11)=1)
  attn = softmax(scores); z = attn@v; o = z@wo+bo
  l1 = x + LN(o);  ffn = relu(l1@w1+b1)@w2+b2;  out = l1 + LN(ffn)

Sharding: 4096 tokens (B=2,S=2048) split 8 ways -> 512 tokens/core.
Cores 0-3 own batch 0, cores 4-7 batch 1. Each core projects K/V for its
own 512-token block; two AllGathers (K then V) within each 4-core batch
group replicate them; both overlap with Q projection and the QK^T+exp
pipeline (scores only need K, attn@V starts once V lands).

Perf notes (from NTFF traces of v1):
 - matmul cost ~ moving-dim cols; consecutive matmuls must hit different
   PSUM banks to pipeline (~195ns/384col vs ~427ns serialized).
 - scores use packed pair K tiles (two heads per 128 partitions) with
   per-head zero-padded Q copies, so contraction is a full 128 rows.
 - mask folded into et post-exp (et' = m*et + (1-m)) so exp scale is a
   compile-time constant and activations batch 2 k-tiles per instr.
 - softmax denominators: ones column in V; 1/den via fast DVE reciprocal,
   broadcast across partitions with a rank-1 matmul.
"""

import math
import os
import sys

import numpy as np

for _p in ("/opt/trn_rl_repo", os.path.expanduser("~/.axon_site/_ro/trn_rl_repo")):
    if os.path.isdir(_p) and _p not in sys.path:
        sys.path.insert(0, _p)

import ml_dtypes  # noqa: E402

BF16 = ml_dtypes.bfloat16

DIM = 768
HEADS = 12
DK = 64
HID = 4 * DIM  # 3072
B, S = 2, 2048
N_CORES = 8
BLK = 512            # tokens per core
NBLK = S // BLK      # 4 blocks per batch
EPS = 1e-5
ISCALE = 1.0 / math.sqrt(DIM)

FT = DIM // 128   # 6 feature tiles
TT = BLK // 128   # 4 token tiles per core block
ST = S // 128     # 16 key tiles per batch
HT = HID // 128   # 24 hidden tiles
NG = ST // 2      # 8 exp groups (2 k-tiles each) per head

_CACHE: dict = {}
DBG = os.environ.get("BASS_DEBUG", "0") == "1"
GP_MASK = os.environ.get("BASS_GP_MASK", "1") == "1"


def _build_program():
    import concourse.bass as bass
    import concourse.mybir as mybir
    import concourse.tile as tile
    from concourse import bacc
    from concourse.masks import make_identity

    f32 = mybir.dt.float32
    bf16 = mybir.dt.bfloat16
    AF = mybir.ActivationFunctionType
    ALU = mybir.AluOpType
    AX = mybir.AxisListType

    nc = bacc.Bacc()

    # ---- per-core DRAM I/O ----
    d_xT = nc.dram_tensor("xT", [128, FT, S], bf16, kind="ExternalInput")
    d_xTb = nc.dram_tensor("xTb", [128, FT, BLK], bf16, kind="ExternalInput")
    d_xb = nc.dram_tensor("xb", [128, TT, DIM], f32, kind="ExternalInput")
    d_m01 = nc.dram_tensor("m01", [128, ST], f32, kind="ExternalInput")
    d_m1m = nc.dram_tensor("m1m", [128, ST], f32, kind="ExternalInput")
    d_wq = nc.dram_tensor("wq", [128, FT, DIM], bf16, kind="ExternalInput")
    d_wk = nc.dram_tensor("wk", [128, FT, DIM], bf16, kind="ExternalInput")
    d_wv = nc.dram_tensor("wv", [128, FT, DIM], bf16, kind="ExternalInput")
    d_wo = nc.dram_tensor("wo", [128, FT, DIM], bf16, kind="ExternalInput")
    d_w1 = nc.dram_tensor("w1", [DIM, HID], bf16, kind="ExternalInput")
    d_w2 = nc.dram_tensor("w2", [HID, DIM], bf16, kind="ExternalInput")
    d_bq = nc.dram_tensor("bq", [128, FT], f32, kind="ExternalInput")
    d_bk = nc.dram_tensor("bk", [128, FT], f32, kind="ExternalInput")
    d_bv = nc.dram_tensor("bv", [DIM], f32, kind="ExternalInput")
    d_bo = nc.dram_tensor("bo", [DIM], f32, kind="ExternalInput")
    d_b1 = nc.dram_tensor("b1", [128, HT], f32, kind="ExternalInput")
    d_b2 = nc.dram_tensor("b2", [DIM], f32, kind="ExternalInput")
    d_g1 = nc.dram_tensor("g1", [DIM], f32, kind="ExternalInput")
    d_bb1 = nc.dram_tensor("bb1", [DIM], f32, kind="ExternalInput")
    d_g2 = nc.dram_tensor("g2", [DIM], f32, kind="ExternalInput")
    d_bb2 = nc.dram_tensor("bb2", [DIM], f32, kind="ExternalInput")
    d_out = nc.dram_tensor("out", [BLK, DIM], f32, kind="ExternalOutput")
    if DBG:
        d_dbg_k = nc.dram_tensor("dbg_k", [128, FT, BLK], bf16, kind="ExternalOutput")
        d_dbg_q = nc.dram_tensor("dbg_q", [128, HEADS, BLK], bf16, kind="ExternalOutput")
        d_dbg_et = nc.dram_tensor("dbg_et", [128, 2, BLK], bf16, kind="ExternalOutput")
        d_dbg_zt = nc.dram_tensor("dbg_zt", [DIM, BLK], bf16, kind="ExternalOutput")
        d_dbg_l1 = nc.dram_tensor("dbg_l1", [128, TT, DIM], f32, kind="ExternalOutput")
        d_dbg_l1t = nc.dram_tensor("dbg_l1t", [DIM, BLK], bf16, kind="ExternalOutput")
        d_dbg_ht = nc.dram_tensor("dbg_ht", [128, HT, BLK], bf16, kind="ExternalOutput")
        d_dbg_z0 = nc.dram_tensor("dbg_z0", [DK + 1, BLK], f32, kind="ExternalOutput")
        d_dbg_rd = nc.dram_tensor("dbg_rd", [1, BLK], f32, kind="ExternalOutput")
        d_dbg_rb = nc.dram_tensor("dbg_rb", [DK, BLK], f32, kind="ExternalOutput")
    d_vb = nc.dram_tensor("v_bounce", [BLK, HEADS * (DK + 1)], bf16)
    d_vs = nc.dram_tensor("v_shared", [S, HEADS * (DK + 1)], bf16)
    RG = [[0, 1, 2, 3], [4, 5, 6, 7]]

    def bcast_ap(handle, n=128):
        ap = handle[:]
        return bass.AP(tensor=ap.tensor, offset=ap.offset, ap=[[0, n]] + list(ap.ap))

    with tile.TileContext(nc) as tc:
        with (
            tc.tile_pool(name="const", bufs=1) as const,
            tc.tile_pool(name="bigres", bufs=1) as big,
        ):
            # ---------- constants ----------
            sb_m01 = const.tile([128, ST], f32)
            nc.gpsimd.dma_start(out=sb_m01, in_=d_m01[:])
            sb_m1m = const.tile([128, ST], f32)
            nc.gpsimd.dma_start(out=sb_m1m, in_=d_m1m[:])
            sb_bq = const.tile([128, FT], f32)
            nc.gpsimd.dma_start(out=sb_bq, in_=d_bq[:])
            sb_bk = const.tile([128, FT], f32)
            nc.gpsimd.dma_start(out=sb_bk, in_=d_bk[:])
            sb_b1 = const.tile([128, HT], f32)
            nc.gpsimd.dma_start(out=sb_b1, in_=d_b1[:])
            bv_bc = const.tile([128, DIM], f32)
            nc.gpsimd.dma_start(out=bv_bc, in_=bcast_ap(d_bv))
            bo_bc = const.tile([128, DIM], f32)
            nc.gpsimd.dma_start(out=bo_bc, in_=bcast_ap(d_bo))
            b2_bc = const.tile([128, DIM], f32)
            nc.gpsimd.dma_start(out=b2_bc, in_=bcast_ap(d_b2))
            g1_bc = const.tile([128, DIM], f32)
            nc.gpsimd.dma_start(out=g1_bc, in_=bcast_ap(d_g1))
            bb1_bc = const.tile([128, DIM], f32)
            nc.gpsimd.dma_start(out=bb1_bc, in_=bcast_ap(d_bb1))
            g2_bc = const.tile([128, DIM], f32)
            nc.gpsimd.dma_start(out=g2_bc, in_=bcast_ap(d_g2))
            bb2_bc = const.tile([128, DIM], f32)
            nc.gpsimd.dma_start(out=bb2_bc, in_=bcast_ap(d_bb2))
            ident_bf = const.tile([128, 128], bf16)
            make_identity(nc, ident_bf[:])
            ones_t = const.tile([128, DK], f32)
            nc.vector.memset(ones_t, 1.0)
            eps_t = const.tile([128, 1], f32)
            nc.vector.memset(eps_t, EPS)

            # ---------- persistent activations ----------
            sb_xblk = big.tile([128, TT, DIM], f32)  # residual x
            sb_l1 = big.tile([128, TT, DIM], f32)
            nc.scalar.dma_start(out=sb_xblk, in_=d_xb[:])

            # ffn-scoped residents (l1T consumed by FFN1, hT by FFN2);
            # opened first so later pools can close in LIFO order
            ffn_res_cm = tc.tile_pool(name="ffn_res", bufs=1)
            ffn_res = ffn_res_cm.__enter__()
            sb_l1T = ffn_res.tile([128, FT, BLK], bf16)
            sb_hT = ffn_res.tile([128, HT, BLK], bf16)

            # wo loaded early (consumed in phase 3)
            wo_cm = tc.tile_pool(name="wo_p", bufs=1)
            wo_p = wo_cm.__enter__()
            w_o = wo_p.tile([128, FT, DIM], bf16)
            nc.scalar.dma_start(out=w_o, in_=d_wo[:])

            # attention-scoped residents (freed after phase 3's O-projection)
            attn_res_cm = tc.tile_pool(name="attn_res", bufs=1)
            attn_res = attn_res_cm.__enter__()
            sb_K = attn_res.tile([128, FT, NBLK, BLK], bf16)  # K^T, feat-major
            sb_Qp = attn_res.tile([128, HEADS, BLK], bf16)  # per-head padded Q^T
            sb_V = attn_res.tile([128, ST, HEADS, DK + 1], bf16)  # V + ones col
            sb_zT = attn_res.tile([128, FT, BLK], bf16)  # z^T normalized
            nc.vector.memset(sb_Qp, 0.0)

            # ============ Phase 1: QKV projections + AllGather(V) ============
            with (
                tc.tile_pool(name="xw", bufs=1) as xw,
                tc.tile_pool(name="xtp", bufs=2) as xtp,
                tc.tile_pool(name="ps1", bufs=4, space="PSUM") as ps1,
                tc.tile_pool(name="ps1v", bufs=4, space="PSUM") as ps1v,
            ):
                sb_xTb = xw.tile([128, FT, BLK], bf16)
                nc.sync.dma_start(out=sb_xTb, in_=d_xTb[:])
                w_v = xw.tile([128, FT, DIM], bf16)
                nc.sync.dma_start(out=w_v, in_=d_wv[:])
                w_k = xw.tile([128, FT, DIM], bf16)
                nc.sync.dma_start(out=w_k, in_=d_wk[:])
                w_q = xw.tile([128, FT, DIM], bf16)
                nc.sync.dma_start(out=w_q, in_=d_wq[:])
                xt_blocks = []
                for b in range(NBLK):
                    xt = xtp.tile([128, FT, BLK], bf16, tag="xt", name=f"xt{b}")
                    nc.sync.dma_start(
                        out=xt, in_=d_xT[:, :, b * BLK : (b + 1) * BLK]
                    )
                    xt_blocks.append(xt)

                # V tok-major for the own block -> bounce -> AG (first: the
                # collective hides behind K/Q/scores work)
                vstage = xw.tile([128, TT, HEADS, DK + 1], bf16, tag="vstage")
                nc.vector.memset(vstage[:, :, :, DK : DK + 1], 1.0)
                vchunks = [(tt, nh) for tt in range(TT) for nh in range(2)]
                for g in range(2):
                    batch = vchunks[g * 4 : (g + 1) * 4]
                    pss = [ps1v.tile([128, 384], f32, tag="vp", name="vp") for _ in batch]
                    for kt in range(FT):
                        for j, (tt, nh) in enumerate(batch):
                            nc.tensor.matmul(
                                pss[j],
                                sb_xTb[:, kt, tt * 128 : (tt + 1) * 128],
                                w_v[:, kt, nh * 384 : (nh + 1) * 384],
                                start=(kt == 0),
                                stop=(kt == FT - 1),
                            )
                    for j, (tt, nh) in enumerate(batch):
                        nc.vector.scalar_tensor_tensor(
                            out=vstage[:, tt, nh * 6 : (nh + 1) * 6, 0:DK],
                            in0=pss[j][:].rearrange("p (h d) -> p h d", d=DK),
                            scalar=1.0,
                            in1=bv_bc[:, nh * 384 : (nh + 1) * 384].rearrange(
                                "p (h d) -> p h d", d=DK
                            ),
                            op0=ALU.mult,
                            op1=ALU.add,
                        )
                nc.scalar.dma_start(
                    out=d_vb[:].rearrange("(t p) (h d) -> p t h d", p=128, d=DK + 1),
                    in_=vstage,
                )
                nc.gpsimd.collective_compute(
                    "AllGather", ALU.bypass, replica_groups=RG,
                    ins=[d_vb[:]], outs=[d_vs[:]],
                )

                # K^T feat-major for the WHOLE batch, computed locally
                # (cheaper than a second collective and never stalls scores)
                for b in range(NBLK):
                    for g in range(2):
                        pss = [ps1.tile([128, BLK], f32, tag="p", name="p") for _ in range(3)]
                        for kt in range(FT):
                            for j in range(3):
                                ft = g * 3 + j
                                nc.tensor.matmul(
                                    pss[j],
                                    w_k[:, kt, ft * 128 : (ft + 1) * 128],
                                    xt_blocks[b][:, kt, :],
                                    start=(kt == 0),
                                    stop=(kt == FT - 1),
                                )
                        for j in range(3):
                            ft = g * 3 + j
                            nc.vector.tensor_scalar_add(
                                sb_K[:, ft, b, :], pss[j], sb_bk[:, ft : ft + 1]
                            )

                # Q^T feat-major; write per-head zero-padded copies
                for g in range(2):
                    pss = [ps1.tile([128, BLK], f32, tag="p", name="p") for _ in range(3)]
                    for kt in range(FT):
                        for j in range(3):
                            ft = g * 3 + j
                            nc.tensor.matmul(
                                pss[j],
                                w_q[:, kt, ft * 128 : (ft + 1) * 128],
                                sb_xTb[:, kt, :],
                                start=(kt == 0),
                                stop=(kt == FT - 1),
                            )
                    for j in range(3):
                        ft = g * 3 + j
                        nc.vector.tensor_scalar_add(
                            sb_Qp[0:64, 2 * ft, :], pss[j][0:64, :],
                            sb_bq[0:64, ft : ft + 1],
                        )
                        nc.vector.tensor_scalar_add(
                            sb_Qp[64:128, 2 * ft + 1, :], pss[j][64:128, :],
                            sb_bq[64:128, ft : ft + 1],
                        )

                if DBG:
                    nc.sync.dma_start(out=d_dbg_k[:], in_=sb_K[:, :, 0, :])
                    nc.sync.dma_start(out=d_dbg_q[:], in_=sb_Qp)
                # unpack the gathered V (waits on the collective)
                nc.sync.dma_start(
                    out=sb_V,
                    in_=d_vs[:].rearrange(
                        "(t p) (h d) -> p t h d", p=128, d=DK + 1
                    ),
                )

            # ============ Phase 2: attention ============
            with (
                tc.tile_pool(name="expp", bufs=20) as expp,
                tc.tile_pool(name="normp", bufs=2) as normp,
                tc.tile_pool(name="ps_sc", bufs=2, space="PSUM") as ps_sc,
                tc.tile_pool(name="ps_z", bufs=2, space="PSUM") as ps_z,
                tc.tile_pool(name="ps_rb", bufs=1, space="PSUM") as ps_rb,
            ):
                et_tiles = {}

                def emit_scores_group(h, g):
                    ps = ps_sc.tile([128, 2, BLK], f32, tag="sc")
                    for j in (0, 1):
                        kt2 = 2 * g + j
                        blk, col = kt2 // NBLK, kt2 % NBLK
                        nc.tensor.matmul(
                            ps[:, j, :],
                            sb_K[:, h // 2, blk, col * 128 : (col + 1) * 128],
                            sb_Qp[:, h, :],
                            start=True, stop=True, skip_group_check=True,
                        )
                    et = expp.tile([128, 2, BLK], bf16, tag="et")
                    nc.scalar.activation(et, ps, AF.Exp, scale=ISCALE)
                    for j in (0, 1):
                        kt2 = 2 * g + j
                        eng = nc.gpsimd if (GP_MASK and j == 1) else nc.vector
                        eng.tensor_scalar(
                            et[:, j, :], et[:, j, :],
                            sb_m01[:, kt2 : kt2 + 1], sb_m1m[:, kt2 : kt2 + 1],
                            op0=ALU.mult, op1=ALU.add,
                        )
                    if DBG and h == 0 and g == 0:
                        nc.sync.dma_start(out=d_dbg_et[:], in_=et)
                    et_tiles[(h, g)] = et

                def emit_attnv_steps(h, zp, g):
                    et = et_tiles.pop((h, g))
                    for j in (0, 1):
                        kt2 = 2 * g + j
                        nc.tensor.matmul(
                            zp,
                            sb_V[:, kt2, h, :],
                            et[:, j, :],
                            start=(kt2 == 0), stop=(kt2 == ST - 1),
                            skip_group_check=True,
                        )

                LOOKAHEAD = 2
                for h in range(LOOKAHEAD):
                    for g in range(NG):
                        emit_scores_group(h, g)
                for h in range(HEADS):
                    zp = ps_z.tile([DK + 1, BLK], f32, tag="z")
                    for g in range(NG):
                        if h + LOOKAHEAD < HEADS:
                            emit_scores_group(h + LOOKAHEAD, g)
                        emit_attnv_steps(h, zp, g)
                    # softmax normalization: z /= den (den = ones-col of V)
                    ht = h // 2
                    ho = (h % 2) * 64
                    if DBG and h == 0:
                        z0dbg = normp.tile([DK + 1, BLK], f32, tag="z0dbg")
                        nc.vector.tensor_copy(z0dbg, zp)
                        nc.sync.dma_start(out=d_dbg_z0[:], in_=z0dbg)
                    den_sb = normp.tile([1, BLK], f32, tag="den_sb")
                    nc.vector.tensor_copy(den_sb, zp[DK : DK + 1, :])
                    rden = normp.tile([128, BLK], f32, tag="rden")
                    nc.vector.reciprocal_approx_fast(rden[0:1, :], den_sb)
                    rb = ps_rb.tile([DK, BLK], f32, tag="rb")
                    nc.tensor.matmul(
                        rb, ones_t[0:1, :], rden[0:1, :],
                        start=True, stop=True, skip_group_check=True,
                    )
                    rbs = normp.tile([DK, BLK], f32, tag="rbs")
                    nc.vector.tensor_copy(rbs, rb)
                    if DBG and h == 0:
                        nc.sync.dma_start(out=d_dbg_rd[:], in_=rden[0:1, :])
                        nc.sync.dma_start(out=d_dbg_rb[:], in_=rbs)
                    nc.vector.tensor_mul(
                        sb_zT[ho : ho + 64, ht, :], zp[0:DK, :], rbs
                    )

            if DBG:
                nc.sync.dma_start(
                    out=d_dbg_zt[:].rearrange("(t p) n -> p t n", p=128), in_=sb_zT
                )

            # ============ Phase 3: O proj + LN1 (+residual) ============
            def layer_norm_to(out_ap, x_ap, g_bc_t, resid_ap, pool):
                s = pool.tile([128, 1], f32, tag="ln_s")
                nc.vector.tensor_reduce(s, x_ap, axis=AX.X, op=ALU.add)
                mean = pool.tile([128, 1], f32, tag="ln_m")
                nc.scalar.mul(mean, s, 1.0 / DIM)
                xc = pool.tile([128, DIM], f32, tag="ln_xc")
                nc.vector.tensor_scalar(xc, x_ap, mean, None, op0=ALU.subtract)
                junk = pool.tile([128, DIM], f32, tag="ln_j")
                var = pool.tile([128, 1], f32, tag="ln_v")
                # (tensor_tensor_reduce crashes the device on this runtime;
                # scalar_tensor_tensor with accum_out works)
                nc.vector.scalar_tensor_tensor(
                    out=junk, in0=xc, scalar=1.0, in1=xc,
                    op0=ALU.mult, op1=ALU.mult, accum_out=var,
                )
                nc.vector.tensor_scalar_mul(var, var, 1.0 / DIM)
                sd = pool.tile([128, 1], f32, tag="ln_sd")
                nc.scalar.activation(sd, var, AF.Sqrt, bias=eps_t[:])
                rstd = pool.tile([128, 1], f32, tag="ln_r")
                nc.vector.reciprocal(rstd, sd)
                t = pool.tile([128, DIM], f32, tag="ln_t")
                nc.vector.tensor_scalar(t, xc, rstd, None, op0=ALU.mult)
                tg = pool.tile([128, DIM], f32, tag="ln_tg")
                nc.vector.tensor_mul(tg, t, g_bc_t)
                nc.vector.tensor_add(out_ap, tg, resid_ap)

            with (
                tc.tile_pool(name="ln1p", bufs=2) as ln1p,
                tc.tile_pool(name="l1bp", bufs=1) as l1bp,
                tc.tile_pool(name="ps_o", bufs=4, space="PSUM") as ps_o,
                tc.tile_pool(name="ps_t", bufs=2, space="PSUM") as ps_t,
            ):
                sb_l1b = l1bp.tile([128, TT, DIM], bf16)
                ochunks = [(tt, nh) for tt in range(TT) for nh in range(2)]
                l1pres = {}
                for gb in range(2):
                    batch = ochunks[gb * 4 : (gb + 1) * 4]
                    pss = [ps_o.tile([128, 384], f32, tag="op", name="op") for _ in batch]
                    for kt in range(FT):
                        for j, (tt, nh) in enumerate(batch):
                            nc.tensor.matmul(
                                pss[j],
                                sb_zT[:, kt, tt * 128 : (tt + 1) * 128],
                                w_o[:, kt, nh * 384 : (nh + 1) * 384],
                                start=(kt == 0),
                                stop=(kt == FT - 1),
                            )
                    for j, (tt, nh) in enumerate(batch):
                        if tt not in l1pres:
                            l1pres[tt] = ln1p.tile([128, DIM], f32, tag="l1pre", name="l1pre")
                        nc.vector.scalar_tensor_tensor(
                            out=l1pres[tt][:, nh * 384 : (nh + 1) * 384],
                            in0=pss[j],
                            scalar=1.0,
                            in1=bo_bc[:, nh * 384 : (nh + 1) * 384],
                            op0=ALU.mult,
                            op1=ALU.add,
                        )
                    for tt, nh in batch:
                        if nh != 1:
                            continue
                        xb1 = ln1p.tile([128, DIM], f32, tag="xb1")
                        nc.vector.tensor_add(xb1, sb_xblk[:, tt, :], bb1_bc)
                        layer_norm_to(
                            sb_l1[:, tt, :], l1pres.pop(tt)[:], g1_bc, xb1, ln1p
                        )
                        nc.scalar.copy(sb_l1b[:, tt, :], sb_l1[:, tt, :])
                        for ft in range(FT):
                            pst = ps_t.tile([128, 128], bf16, tag="tp")
                            nc.tensor.transpose(
                                pst, sb_l1b[:, tt, ft * 128 : (ft + 1) * 128],
                                ident_bf[:],
                            )
                            nc.vector.tensor_copy(
                                sb_l1T[:, ft, tt * 128 : (tt + 1) * 128], pst
                            )

            if DBG:
                nc.sync.dma_start(out=d_dbg_l1[:], in_=sb_l1)
                nc.sync.dma_start(
                    out=d_dbg_l1t[:].rearrange("(t p) n -> p t n", p=128),
                    in_=sb_l1T,
                )
            attn_res_cm.__exit__(None, None, None)
            wo_cm.__exit__(None, None, None)

            # ============ Phase 4: FFN1 ============
            w2_cm = tc.tile_pool(name="w2_p", bufs=1)
            w2_p = w2_cm.__enter__()
            w1_cm = tc.tile_pool(name="w1_p", bufs=1)
            w1_p = w1_cm.__enter__()
            w1_t = []
            for kt in range(FT):
                wt = w1_p.tile([128, HID], bf16, tag=f"w1_{kt}", name=f"w1_{kt}")
                nc.sync.dma_start(out=wt, in_=d_w1[kt * 128 : (kt + 1) * 128, :])
                w1_t.append(wt)
            w2_t = []
            for kt in range(HT):
                wt = w2_p.tile([128, DIM], bf16, tag=f"w2_{kt}", name=f"w2_{kt}")
                nc.sync.dma_start(out=wt, in_=d_w2[kt * 128 : (kt + 1) * 128, :])
                w2_t.append(wt)

            with tc.tile_pool(name="ps_f1", bufs=4, space="PSUM") as ps_f1:
                for g in range(0, HT, 3):
                    pss = [ps_f1.tile([128, BLK], f32, tag="f1", name="f1") for _ in range(3)]
                    for kt in range(FT):
                        for j in range(3):
                            ht2 = g + j
                            nc.tensor.matmul(
                                pss[j],
                                w1_t[kt][:, ht2 * 128 : (ht2 + 1) * 128],
                                sb_l1T[:, kt, :],
                                start=(kt == 0),
                                stop=(kt == FT - 1),
                            )
                    for j in range(3):
                        ht2 = g + j
                        # relu(x + b1) on DVE: (x add b1) max 0
                        nc.vector.tensor_scalar(
                            sb_hT[:, ht2, :], pss[j], sb_b1[:, ht2 : ht2 + 1], 0.0,
                            op0=ALU.add, op1=ALU.max,
                        )
            w1_cm.__exit__(None, None, None)

            if DBG:
                nc.sync.dma_start(out=d_dbg_ht[:], in_=sb_hT)

            # ============ Phase 5: FFN2 + LN2 + out ============
            with (
                tc.tile_pool(name="ln2p", bufs=2) as ln2p,
                tc.tile_pool(name="outp", bufs=3) as outp,
                tc.tile_pool(name="ps_f2", bufs=4, space="PSUM") as ps_f2,
            ):
                out_r = d_out[:].rearrange("(t p) d -> p t d", p=128)
                for tt in range(TT):
                    f2pre = ln2p.tile([128, DIM], f32, tag="f2pre")
                    for nh in range(2):
                        ps = ps_f2.tile([128, 384], f32, tag="f2")
                        for kt in range(HT):
                            nc.tensor.matmul(
                                ps,
                                sb_hT[:, kt, tt * 128 : (tt + 1) * 128],
                                w2_t[kt][:, nh * 384 : (nh + 1) * 384],
                                start=(kt == 0),
                                stop=(kt == HT - 1),
                            )
                        nc.vector.scalar_tensor_tensor(
                            out=f2pre[:, nh * 384 : (nh + 1) * 384],
                            in0=ps,
                            scalar=1.0,
                            in1=b2_bc[:, nh * 384 : (nh + 1) * 384],
                            op0=ALU.mult,
                            op1=ALU.add,
                        )
                    l1b2 = ln2p.tile([128, DIM], f32, tag="l1b2")
                    nc.vector.tensor_add(l1b2, sb_l1[:, tt, :], bb2_bc)
                    o_sb = outp.tile([128, DIM], f32, tag="osb")
                    layer_norm_to(o_sb[:], f2pre[:], g2_bc, l1b2, ln2p)
                    nc.sync.dma_start(out=out_r[:, tt, :], in_=o_sb)
            w2_cm.__exit__(None, None, None)
            ffn_res_cm.__exit__(None, None, None)

    return nc


def _get_nc(finalized=True):
    if "nc" not in _CACHE:
        _CACHE["nc"] = _build_program()
    nc = _CACHE["nc"]
    if finalized and not nc.is_finalized():
        nc.finalize()
    return nc


def _ptile(a, p=128):
    """[T*p, N...] -> [p, T, N...] partition-tiled contiguous layout."""
    a = np.asarray(a)
    t = a.shape[0] // p
    return np.ascontiguousarray(
        a.reshape(t, p, *a.shape[1:]).transpose(1, 0, *range(2, a.ndim + 1))
    )


def make_in_maps(inputs: dict) -> list:
    x = np.asarray(inputs["x_n"], np.float32).reshape(B, S, DIM)
    mask = np.asarray(inputs["mask"]).reshape(B, S)
    w = {
        k: _ptile(np.asarray(inputs[k], np.float32).astype(BF16))
        for k in ("wq", "wk", "wv", "wo")
    }
    w["w1"] = np.ascontiguousarray(np.asarray(inputs["w1"], np.float32).astype(BF16))
    w["w2"] = np.ascontiguousarray(np.asarray(inputs["w2"], np.float32).astype(BF16))
    vecs = {
        "bv": inputs["bv"], "bo": inputs["bo"], "b2": inputs["b2"],
        "g1": inputs["ln1_g"], "bb1": inputs["ln1_b"],
        "g2": inputs["ln2_g"], "bb2": inputs["ln2_b"],
    }
    vecs = {k: np.ascontiguousarray(np.asarray(v, np.float32)) for k, v in vecs.items()}
    vecs["bq"] = _ptile(np.asarray(inputs["bq"], np.float32))
    vecs["bk"] = _ptile(np.asarray(inputs["bk"], np.float32))
    vecs["b1"] = _ptile(np.asarray(inputs["b1"], np.float32))
    xT_b = [_ptile(np.ascontiguousarray(x[b].T).astype(BF16)) for b in range(B)]
    m01_b = []
    for b in range(B):
        m01 = (mask[b] != 0).astype(np.float32)
        m01_b.append((_ptile(m01), _ptile(np.float32(1.0) - m01)))
    in_maps = []
    for c in range(N_CORES):
        b, blk = c // NBLK, c % NBLK
        xblk = x[b][blk * BLK : (blk + 1) * BLK]
        m = {
            "xT": xT_b[b],
            "xTb": _ptile(np.ascontiguousarray(xblk.T).astype(BF16)),
            "xb": _ptile(xblk),
            "m01": m01_b[b][0],
            "m1m": m01_b[b][1],
        }
        m.update(w)
        m.update(vecs)
        in_maps.append(m)
    return in_maps


def assemble(per_core_out: list) -> np.ndarray:
    blocks = [np.asarray(o, np.float32) for o in per_core_out]
    full = np.concatenate(blocks, axis=0).reshape(B, S, DIM)
    return full


def kernel(**inputs) -> np.ndarray:
    from concourse.bass_utils import run_bass_kernel_spmd

    nc = _get_nc()
    in_maps = make_in_maps(inputs)
    res = run_bass_kernel_spmd(nc, in_maps, list(range(N_CORES)))
    return assemble([r["out"] for r in res.results])
